# revision 1
# baseline (speedup 1.0000x reference)
"""Trainium2 Bass kernel for nn_Decoder (recursive tree GRU decoder).

Self-contained: builds + compiles + runs a Bass/Tile kernel SPMD on 8
NeuronCores, pure data-parallel over the batch dim.

Math (per batch element, mirroring the reference):
  hidden0 = z @ z2h_w + z2h_b
  preorder tree of depth DEPTH / arity ARITY; at each node v:
    pred_v = h_v @ h2o_w + h2o_b            (output, pre-softmax logits)
    probs_v = softmax(pred_v)
    child1 = GRU_anc(probs_v, h_v)
    f = child1; for each later sibling:
      f = GRU_frat(probs_prev_child, f)
      child_k = tanh(f @ uf_w + uf_b + h_v @ ua_w + ua_b)

On-chip layout: feature-major [feat(partitions), batch(free)], batch tile
BT=512, 4 tiles processed in lockstep per "quad group" so the O=32 pred /
softmax tensors pack 4 tiles into 128 partitions.  Matmuls run in float32r
(single-pass PE, ~4x faster than fp32, same observed precision ~2e-4).
GRU gates use tanh only (sigmoid(x) = (tanh(x/2)+1)/2) so exp+tanh share one
ACT table set (no 2.7us table switches).
"""

import numpy as np

import concourse.bass as bass
import concourse.mybir as mybir
from concourse import tile, masks
from concourse.bass_utils import run_bass_kernel_spmd

F32 = mybir.dt.float32
F32R = mybir.dt.float32r
BF16 = mybir.dt.bfloat16
AF = mybir.ActivationFunctionType
ALU = mybir.AluOpType

B, I, H, O = 32768, 128, 128, 32
N_CORES = 8
BT = 512          # batch tile (free dim of one matmul / PSUM bank)
NT = 4            # tiles per quad group (O*NT = 128 partitions)
B_CORE = B // N_CORES          # 4096
NQ = B_CORE // (BT * NT)       # 2 quad groups per core

_PE_OPS = ("InstMatmult", "InstLdweights", "InstMatmultMx")


def _split_multi_waits(nc):
    """This container's walrus accepts at most 1 embedded sem wait on most
    instructions (0 on self-loading matmuls) and <=2 on a standalone
    EventSemaphore.  Tile emits multi-waits; split them."""
    for f in nc.m.functions:
        for bb in f.blocks:
            insts = bb.instructions
            new = []
            changed = False
            for ins in insts:
                si = ins.sync_info
                ow = list(si.on_wait) if si is not None and si.on_wait else []
                movable = [w for w in ow if w.wait_reg is None]
                fixed = [w for w in ow if w.wait_reg is not None]
                opc = type(ins).__name__
                limit = 0 if opc in _PE_OPS else 1
                limit = max(0, limit - len(fixed))
                if len(movable) > limit:
                    keep = movable[:limit]
                    move = movable[limit:]
                    for i in range(0, len(move), 2):
                        ev = mybir.InstEventSemaphore(
                            name=f"{ins.name}-wsp{i}",
                            ins=[],
                            outs=[],
                            sync_info=mybir.SyncInfo(
                                on_wait=move[i : i + 2], on_update=[]
                            ),
                        )
                        ev.engine = ins.engine
                        new.append(ev)
                    upd = list(si.on_update) if si.on_update else []
                    ins.sync_info = mybir.SyncInfo(on_wait=fixed + keep, on_update=upd)
                    changed = True
                new.append(ins)
            if changed:
                bb.instructions = new


def _n_nodes(depth, arity):
    n, level = 0, 1
    for _ in range(depth + 1):
        n += level
        level *= arity
    return n


def build(depth, arity, dt_act=F32R, loop_n=1):
    """Build the per-core Bass module.  Returns (nc, n_nodes)."""
    nn_ = _n_nodes(depth, arity)
    nc = bass.Bass(trn_type="TRN2")

    # ---- DRAM I/O (per-core shapes) ----
    z_d = nc.dram_tensor("z", [B_CORE, I], F32, kind="ExternalInput")
    # weights, host-preprocessed (see _prep_weights)
    w_misc_d = nc.dram_tensor("w_misc", [128, 128 * 2 + O * NT * 4], dt_act, kind="ExternalInput")
    # w_misc columns: [z2h(128) | S_rep(128) | h2o_pad(512)]
    w_gru_d = nc.dram_tensor("w_gru", [128, 2 * (3 * NT * 128 + 3 * 128)], dt_act, kind="ExternalInput")
    # per gru g in (anc, frat): [wi_pad k=0..2,t=0..3 (12*128) | wh0,wh1,wh2half (3*128)]
    w_u_d = nc.dram_tensor("w_u", [128, 256], dt_act, kind="ExternalInput")  # [uf | ua]
    ident_d = nc.dram_tensor("ident", [128, 128], F32, kind="ExternalInput")
    # HBM layout [node, qg, p, c, t, o]; host transposes to [node, batch, 1, O]
    out_d = nc.dram_tensor("out", [nn_, NQ, 128, 4, NT, O], F32, kind="ExternalOutput")

    GRU_STRIDE = 3 * NT * 128 + 3 * 128

    with tile.TileContext(nc) as tc:
        with (
            tc.tile_pool(name="wp", bufs=1) as wp,
            tc.tile_pool(name="hp", bufs=1) as hp,
            tc.tile_pool(name="prp", bufs=1) as prp,
            tc.tile_pool(name="tp", bufs=2) as tp,
            tc.tile_pool(name="trzp", bufs=3) as trzp,
            tc.tile_pool(name="pp", bufs=3, space="PSUM") as pp,
            tc.tile_pool(name="pb", bufs=2, space="PSUM") as pb,
        ):
            # ---- load weights once ----
            w_misc = wp.tile([128, 128 * 2 + O * NT * 4], dt_act, tag="w_misc")
            w_gru = wp.tile([128, 2 * GRU_STRIDE], dt_act, tag="w_gru")
            w_u = wp.tile([128, 256], dt_act, tag="w_u")
            ident = wp.tile([128, 128], F32, tag="ident")
            nc.sync.dma_start(w_misc[:], w_misc_d[:])
            nc.sync.dma_start(w_gru[:], w_gru_d[:])
            nc.sync.dma_start(w_u[:], w_u_d[:])
            nc.sync.dma_start(ident[:], ident_d[:])

            w_z2h = w_misc[:, 0:128]
            w_S = w_misc[:, 128:256]
            def w_h2o(t):
                return w_misc[:, 256 + t * 128 : 256 + (t + 1) * 128]
            def w_gi(g, k, t):
                base = g * GRU_STRIDE + (k * NT + t) * 128
                return w_gru[:, base : base + 128]
            def w_gh(g, k):
                base = g * GRU_STRIDE + 3 * NT * 128 + k * 128
                return w_gru[:, base : base + 128]
            w_uf = w_u[:, 0:128]
            w_ua = w_u[:, 128:256]

            from contextlib import ExitStack
            _ls = ExitStack()
            if loop_n > 1:
                _ls.enter_context(tc.For_i(0, loop_n, 1))

            def qg_gen(qg):
                node_idx = [0]

                # ---- hidden0 = z @ z2h_w  (feature-major) ----
                h0 = []
                for t in range(NT):
                    base = qg * (NT * BT) + t * BT
                    zbm = tp.tile([128, BT], F32, tag="zpair")
                    nc.sync.dma_start(
                        zbm[:].rearrange("p (c f) -> p c f", c=4, f=128),
                        z_d[base : base + BT, :].rearrange("(c p) f -> p c f", c=4, p=128),
                    )
                    zT_ps = pb.tile([128, BT], F32, tag="pb1")
                    for c in range(4):
                        nc.tensor.transpose(
                            zT_ps[:, c * 128 : (c + 1) * 128],
                            zbm[:, c * 128 : (c + 1) * 128],
                            ident[:],
                        )
                    zT = tp.tile([128, BT], dt_act, tag="zpair")
                    nc.scalar.copy(zT[:], zT_ps[:])
                    h_ps = pb.tile([128, BT], F32, tag="pb1")
                    nc.tensor.matmul(h_ps[:], w_z2h, zT[:], start=True, stop=True)
                    ht = hp.tile([128, BT], dt_act, tag=f"h_q{qg}_d{depth}_t{t}")
                    nc.scalar.copy(ht[:], h_ps[:])
                    h0.append(ht)

                def pred_softmax(h, d, need_probs):
                    n = node_idx[0]
                    node_idx[0] += 1
                    pred_ps = pb.tile([128, BT], F32, tag="pb1")
                    for t in range(NT):
                        nc.tensor.matmul(
                            pred_ps[:], w_h2o(t), h[t][:],
                            start=(t == 0), stop=(t == NT - 1),
                        )
                    # output path (always fp32)
                    pred_sb = tp.tile([128, BT], F32, tag="pout")
                    nc.scalar.copy(pred_sb[:], pred_ps[:])
                    tr_ps = pb.tile([128, BT], F32, tag="pb1")
                    for c in range(4):
                        nc.tensor.transpose(
                            tr_ps[:, c * 128 : (c + 1) * 128],
                            pred_sb[:, c * 128 : (c + 1) * 128],
                            ident[:],
                        )
                    out_sb = tp.tile([128, BT], F32, tag="pout")
                    nc.scalar.copy(out_sb[:], tr_ps[:])
                    nc.sync.dma_start(
                        out_d[n, qg].rearrange("p c t o -> p (c t o)"),
                        out_sb[:],
                    )
                    if not need_probs:
                        return None
                    # softmax (no max-subtraction; logits are small)
                    exp_sb = tp.tile([128, BT], dt_act, tag="exp_sb")
                    nc.scalar.activation(exp_sb[:], pred_ps[:], AF.Exp, bias=0.0, scale=1.0)
                    sums_ps = pb.tile([128, BT], F32, tag="pb1")
                    nc.tensor.matmul(sums_ps[:], w_S, exp_sb[:], start=True, stop=True)
                    rbc = tp.tile([128, BT], F32, tag="scr")
                    nc.vector.reciprocal(rbc[:], sums_ps[:])
                    probs = prp.tile([128, BT], dt_act, tag=f"probs_q{qg}_d{d}")
                    nc.vector.tensor_tensor(out=probs[:], in0=exp_sb[:], in1=rbc[:], op=ALU.mult)
                    return probs

                def gru(g, probs, h, d):
                    hnew = []
                    for t in range(NT):
                        AB = pp.tile([128, 2 * BT], F32, tag="gpair")
                        nc.tensor.matmul(AB[:, 0:BT], w_gi(g, 0, t), probs[:], start=True, stop=False)
                        nc.tensor.matmul(AB[:, 0:BT], w_gh(g, 0), h[t][:], start=False, stop=True)
                        nc.tensor.matmul(AB[:, BT:], w_gi(g, 1, t), probs[:], start=True, stop=False)
                        nc.tensor.matmul(AB[:, BT:], w_gh(g, 1), h[t][:], start=False, stop=True)
                        CD = pp.tile([128, 2 * BT], F32, tag="gpair")
                        nc.tensor.matmul(CD[:, 0:BT], w_gi(g, 2, t), probs[:], start=True, stop=True)
                        nc.tensor.matmul(CD[:, BT:], w_gh(g, 2), h[t][:], start=True, stop=True)
                        trz = trzp.tile([128, 2 * BT], F32, tag="trz")
                        nc.scalar.activation(trz[:], AB[:], AF.Tanh, bias=0.0, scale=0.5)
                        m_sb = tp.tile([128, BT], F32, tag="scr")
                        nc.vector.scalar_tensor_tensor(
                            out=m_sb[:], in0=trz[:, 0:BT], scalar=1.0, in1=CD[:, BT:],
                            op0=ALU.add, op1=ALU.mult,
                        )
                        na = tp.tile([128, BT], F32, tag="na")
                        nc.vector.tensor_tensor(out=na[:], in0=m_sb[:], in1=CD[:, 0:BT], op=ALU.add)
                        nn_t = tp.tile([128, BT], F32, tag="nn_t")
                        nc.scalar.activation(nn_t[:], na[:], AF.Tanh, bias=0.0, scale=1.0)
                        s_sb = tp.tile([128, BT], F32, tag="s_sb")
                        nc.vector.tensor_tensor(
                            out=s_sb[:], in0=h[t][:].bitcast(F32) if dt_act == F32R else h[t][:],
                            in1=nn_t[:], op=ALU.subtract,
                        )
                        u1 = tp.tile([128, BT], F32, tag="u1")
                        nc.vector.scalar_tensor_tensor(
                            out=u1[:], in0=trz[:, BT:], scalar=1.0, in1=s_sb[:],
                            op0=ALU.add, op1=ALU.mult,
                        )
                        hn = hp.tile([128, BT], dt_act, tag=f"h_q{qg}_d{d}_t{t}")
                        nc.vector.scalar_tensor_tensor(
                            out=hn[:], in0=u1[:], scalar=0.5, in1=nn_t[:],
                            op0=ALU.mult, op1=ALU.add,
                        )
                        hnew.append(hn)
                    return hnew

                def u_stage(hf, h, d):
                    h2 = []
                    for t in range(NT):
                        U_ps = pb.tile([128, BT], F32, tag="pb1")
                        nc.tensor.matmul(U_ps[:], w_uf, hf[t][:], start=True, stop=False)
                        nc.tensor.matmul(U_ps[:], w_ua, h[t][:], start=False, stop=True)
                        ht = hp.tile([128, BT], dt_act, tag=f"h_q{qg}_d{d}_t{t}")
                        nc.scalar.activation(ht[:], U_ps[:], AF.Tanh, bias=0.0, scale=1.0)
                        h2.append(ht)
                    return h2

                def rec(h, d, need_probs):
                    probs = pred_softmax(h, d, need_probs or d > 0)
                    yield
                    if d == 0:
                        return probs
                    h1 = gru(0, probs, h, d - 1)
                    yield
                    probs_f = yield from rec(h1, d - 1, arity > 1)
                    hf = h1
                    for s in range(arity - 1):
                        hf = gru(1, probs_f, hf, d - 1)
                        yield
                        h2 = u_stage(hf, h, d - 1)
                        probs_f = yield from rec(h2, d - 1, s < arity - 2)
                    return probs

                yield from rec(h0, depth, False)

            gens = [qg_gen(qg) for qg in range(NQ)]
            live = list(gens)
            while live:
                for g in list(live):
                    try:
                        next(g)
                    except StopIteration:
                        live.remove(g)

            _ls.close()

    _split_multi_waits(nc)
    return nc, nn_


def _prep_weights(inputs, dt_np=np.float32):
    """Host-side weight preprocessing into the packed DRAM layouts."""
    f = lambda x: np.asarray(x, dtype=np.float32)
    z2h_w = f(inputs["z2h_w"])            # [I, H]
    h2o_w = f(inputs["h2o_w"])            # [H, O]
    S = np.zeros((128, 128), np.float32)
    for t in range(NT):
        S[t * O : (t + 1) * O, t * O : (t + 1) * O] = 1.0
    h2o_pad = np.zeros((128, NT * 128), np.float32)
    for t in range(NT):
        h2o_pad[:, t * 128 + t * O : t * 128 + (t + 1) * O] = h2o_w
    w_misc = np.concatenate([z2h_w, S, h2o_pad], axis=1)

    blocks = []
    for name in ("anc", "frat"):
        wi = f(inputs[f"{name}_wi"])      # [3, O, H]
        wh = f(inputs[f"{name}_wh"])      # [3, H, H]
        wi_pad = np.zeros((128, 3 * NT * 128), np.float32)
        for k in range(3):
            for t in range(NT):
                wi_pad[t * O : (t + 1) * O, (k * NT + t) * 128 : (k * NT + t + 1) * 128] = wi[k]
        wh_cat = np.concatenate([wh[0], wh[1], 0.5 * wh[2]], axis=1)
        blocks.append(np.concatenate([wi_pad, wh_cat], axis=1))
    w_gru = np.concatenate(blocks, axis=1)

    w_u = np.concatenate([f(inputs["uf_w"]), f(inputs["ua_w"])], axis=1)
    ident = np.eye(128, dtype=np.float32)
    return {
        "w_misc": w_misc.astype(dt_np),
        "w_gru": w_gru.astype(dt_np),
        "w_u": w_u.astype(dt_np),
        "ident": ident,
    }


_BUILD_CACHE = {}


def _get_built(depth, arity):
    key = (depth, arity)
    if key not in _BUILD_CACHE:
        _BUILD_CACHE[key] = build(depth, arity)
    return _BUILD_CACHE[key]


def kernel(**inputs) -> np.ndarray:
    depth = int(np.asarray(inputs["depth"]))
    arity = int(np.asarray(inputs["arity"]))
    for bname in ("z2h_b", "h2o_b", "anc_bi", "anc_bh", "frat_bi", "frat_bh", "ua_b", "uf_b"):
        if bname in inputs and np.any(np.asarray(inputs[bname])):
            raise NotImplementedError(f"nonzero bias {bname} not supported")

    nc, nn_ = _get_built(depth, arity)
    w = _prep_weights(inputs)
    z = np.asarray(inputs["z"], dtype=np.float32).reshape(B, I)

    in_maps = []
    for c in range(N_CORES):
        im = dict(w)
        im["z"] = np.ascontiguousarray(z[c * B_CORE : (c + 1) * B_CORE])
        in_maps.append(im)

    res = run_bass_kernel_spmd(nc, in_maps, core_ids=list(range(N_CORES)))
    outs = []
    for c in range(N_CORES):
        o = np.asarray(res.results[c]["out"])  # [nn, NQ, p, c, t, o]
        o = o.transpose(0, 1, 4, 3, 2, 5).reshape(nn_, B_CORE, 1, O)
        outs.append(o)
    return np.concatenate(outs, axis=1)  # [nn, B, 1, O]


if __name__ == "__main__":
    # smoke test with random inputs
    rng = np.random.default_rng(0)
    ins = {
        "z": rng.standard_normal((B, 1, I)).astype(np.float32),
        "z2h_w": rng.standard_normal((I, H)).astype(np.float32) * 0.08,
        "z2h_b": np.zeros(H, np.float32),
        "h2o_w": rng.standard_normal((H, O)).astype(np.float32) * 0.1,
        "h2o_b": np.zeros(O, np.float32),
        "anc_wi": rng.standard_normal((3, O, H)).astype(np.float32) * 0.1,
        "anc_wh": rng.standard_normal((3, H, H)).astype(np.float32) * 0.08,
        "anc_bi": np.zeros((3, H), np.float32),
        "anc_bh": np.zeros((3, H), np.float32),
        "frat_wi": rng.standard_normal((3, O, H)).astype(np.float32) * 0.1,
        "frat_wh": rng.standard_normal((3, H, H)).astype(np.float32) * 0.08,
        "frat_bi": np.zeros((3, H), np.float32),
        "frat_bh": np.zeros((3, H), np.float32),
        "ua_w": rng.standard_normal((H, H)).astype(np.float32) * 0.08,
        "ua_b": np.zeros(H, np.float32),
        "uf_w": rng.standard_normal((H, H)).astype(np.float32) * 0.08,
        "uf_b": np.zeros(H, np.float32),
        "depth": np.int64(2),
        "arity": np.int64(2),
    }
    out = kernel(**ins)
    print("out shape:", out.shape, "finite:", np.isfinite(out).all())

